# revision 8
# baseline (speedup 1.0000x reference)
"""Multi-head attention (B=4, S=2048, C=768, H=8, HD=96) on 8 TRN2 NeuronCores.

Strategy: tensor-parallel by head — one head per core.
  - Host pre-transposes x -> xT [C, T] so the QKV matmul's contraction dim (C)
    lands on SBUF partitions with contiguous DMA (no on-device x transposes).
  - Per core h: qkv = x @ Wqkv_h.T via PE (fp32r, full rate), RoPE on DVE in
    token-major layout, PE-transpose q/k to [HD, tokens] for attention.
  - Attention per (b, q-tile): scores.T [k,q] = kT.T @ qT on PE, exp on ACT
    (scale folded in, no max-subtraction needed: scores ~ N(0,1)), P.T stays in
    SBUF and feeds PV directly: out.T [HD+1, q] = v_aug.T @ P.T where v is
    augmented with a ones column so row HD accumulates the softmax denominator.
  - Normalize with a reciprocal in token-parallel layout + PE ones-broadcast.
  - AllToAll redistributes head-major outputs to token-sliced channel-major
    layout; each core then computes the full output projection for its 1024
    tokens as y.T = Wproj @ out_all.T (weights host-pre-transposed), + bias.
  - Host assembles y from the 8 per-core y.T slices.
"""

import numpy as np
from contextlib import ExitStack

import concourse.bass as bass
from concourse import bacc
import concourse.tile as tile
from concourse import mybir
from concourse.bass_utils import run_bass_kernel_spmd
from concourse.masks import make_identity

B, S, C, H, HD = 4, 2048, 768, 8, 96
T = B * S            # 8192 tokens
NCORES = 8
TSLICE = T // NCORES  # 1024 tokens per core for the projection
KC = C // 128        # 6 contraction chunks of 128
F32 = mybir.dt.float32
F32R = mybir.dt.float32r

USE_F32R = True


def _r(ap):
    return ap.bitcast(F32R) if USE_F32R else ap


def build_nc():
    nc = bacc.Bacc(None, num_devices=NCORES)

    xT = nc.declare_dram_parameter("xT", [C, T], F32, isOutput=False)
    wqkvT = nc.declare_dram_parameter("wqkvT", [C, 3 * HD], F32, isOutput=False)
    wprojT = nc.declare_dram_parameter("wprojT", [C, C], F32, isOutput=False)
    cosd = nc.declare_dram_parameter("cos", [S, HD], F32, isOutput=False)
    snd = nc.declare_dram_parameter("s", [S, HD], F32, isOutput=False)
    biasd = nc.declare_dram_parameter("bias", [128, KC], F32, isOutput=False)
    outd = nc.declare_dram_parameter("out", [C, TSLICE], F32, isOutput=True)

    a2a_in = nc.dram_tensor("a2a_in", [C, TSLICE], F32)
    a2a_out = nc.dram_tensor("a2a_out", [C, TSLICE], F32)

    SCALE = HD ** -0.5
    MULT = mybir.AluOpType.mult
    ADD = mybir.AluOpType.add
    EXP = mybir.ActivationFunctionType.Exp
    IDENT = mybir.ActivationFunctionType.Identity

    with tile.TileContext(nc, num_cores=NCORES) as tc, ExitStack() as ctx:
        const = ctx.enter_context(tc.tile_pool(name="const", bufs=1))
        xtp = ctx.enter_context(tc.tile_pool(name="xtp", bufs=3))
        ropep = ctx.enter_context(tc.tile_pool(name="ropep", bufs=3))
        Pp = ctx.enter_context(tc.tile_pool(name="Pp", bufs=3))
        nrm = ctx.enter_context(tc.tile_pool(name="nrm", bufs=3))
        rcp = ctx.enter_context(tc.tile_pool(name="rcp", bufs=2))
        yp = ctx.enter_context(tc.tile_pool(name="yp", bufs=2))

        ps288 = ctx.enter_context(tc.tile_pool(name="ps288", bufs=2, space="PSUM"))
        pstp = ctx.enter_context(tc.tile_pool(name="pstp", bufs=2, space="PSUM"))
        psc = ctx.enter_context(tc.tile_pool(name="psc", bufs=2, space="PSUM"))
        psb = ctx.enter_context(tc.tile_pool(name="psb", bufs=1, space="PSUM"))
        psacc = ctx.enter_context(tc.tile_pool(name="psacc", bufs=1, space="PSUM"))

        # --- constants ---
        wq_sb = const.tile([128, KC, 3 * HD], F32)
        nc.sync.dma_start(_r(wq_sb), _r(wqkvT.ap().rearrange("(kc p) n -> p kc n", p=128)))
        wp_sb = const.tile([128, KC, C], F32)
        nc.sync.dma_start(_r(wp_sb), _r(wprojT.ap().rearrange("(kc p) n -> p kc n", p=128)))
        cos_sb = const.tile([128, 16, HD], F32)
        nc.sync.dma_start(cos_sb, cosd.ap().rearrange("(kt p) c -> p kt c", p=128))
        s_sb = const.tile([128, 16, HD], F32)
        nc.sync.dma_start(s_sb, snd.ap().rearrange("(kt p) c -> p kt c", p=128))
        bias_sb = const.tile([128, KC], F32)
        nc.sync.dma_start(bias_sb, biasd.ap())
        ident = const.tile([128, 128], F32)
        make_identity(nc, ident)
        ones_sb = const.tile([128, HD], F32)
        nc.vector.memset(ones_sb, 1.0)
        rpad = const.tile([128, 512], F32)
        nc.vector.memset(rpad, 0.0)

        # persistent ping/pong per-batch q/k (transposed, channel-padded) and v
        qT = [const.tile([128, S], F32, name=f"qT{i}") for i in range(2)]
        kT = [const.tile([128, S], F32, name=f"kT{i}") for i in range(2)]
        vA = [const.tile([128, 16, HD + 1], F32, name=f"vA{i}") for i in range(2)]
        for i in range(2):
            nc.vector.memset(qT[i][HD:128, :], 0.0)
            nc.vector.memset(kT[i][HD:128, :], 0.0)
            nc.vector.memset(vA[i][:, :, HD:HD + 1], 1.0)

        xTv = xT.ap().rearrange("(kc p) t -> p kc t", p=128)  # [128, KC, T]

        for b in range(B):
            q_b, k_b, v_b = qT[b % 2], kT[b % 2], vA[b % 2]

            # ---- phase 1: qkv + rope for batch b ----
            for g in range(16):
                tok0 = b * S + g * 128
                xt = xtp.tile([128, KC, 128], F32)
                nc.sync.dma_start(_r(xt), _r(xTv[:, :, tok0:tok0 + 128]))
                pq = ps288.tile([128, 3 * HD], F32)
                for kc in range(KC):
                    nc.tensor.matmul(
                        pq, _r(xt[:, kc, :]), _r(wq_sb[:, kc, :]),
                        start=(kc == 0), stop=(kc == KC - 1),
                    )
                cs = cos_sb[:, g, :]
                sn = s_sb[:, g, :]
                sn3 = sn.rearrange("p (a two) -> p a two", two=2)
                for off, dstT in ((0, q_b), (HD, k_b)):
                    src = pq[:, off:off + HD]
                    src3 = src.rearrange("p (a two) -> p a two", two=2)
                    t1 = ropep.tile([128, HD], F32, tag="rope_t1")
                    nc.vector.tensor_tensor(t1, src, cs, MULT)
                    t2 = ropep.tile([128, HD], F32, tag="rope_t2")
                    t23 = t2.rearrange("p (a two) -> p a two", two=2)
                    nc.vector.tensor_tensor(t23[:, :, 0], src3[:, :, 1], sn3[:, :, 0], MULT)
                    nc.vector.tensor_tensor(t23[:, :, 1], src3[:, :, 0], sn3[:, :, 1], MULT)
                    t3 = ropep.tile([128, HD], F32, tag="rope_t3")
                    nc.vector.tensor_tensor(t3, t1, t2, ADD)
                    ptp = pstp.tile([HD, 128], F32)
                    nc.tensor.transpose(ptp, t3, ident)
                    nc.vector.tensor_copy(out=_r(dstT[0:HD, g * 128:(g + 1) * 128]), in_=ptp)
                nc.vector.tensor_copy(out=_r(v_b[:, g, 0:HD]), in_=pq[:, 2 * HD:3 * HD])

            # ---- phase 2: attention for batch b ----
            for qt in range(4):
                acc = psacc.tile([HD + 1, 512], F32)
                for kt in range(16):
                    sc = psc.tile([128, 512], F32, tag="sc512")
                    nc.tensor.matmul(
                        sc, _r(k_b[:, kt * 128:(kt + 1) * 128]),
                        _r(q_b[:, qt * 512:(qt + 1) * 512]),
                        start=True, stop=True,
                    )
                    Pt = Pp.tile([128, 512], F32)
                    nc.scalar.activation(_r(Pt), sc, EXP, scale=SCALE)
                    nc.tensor.matmul(
                        acc, _r(v_b[:, kt, :]), _r(Pt),
                        start=(kt == 0), stop=(kt == 15),
                    )
                # normalize: recip of denominators (row HD of acc), PE-broadcast
                dnrow = rcp.tile([1, 512], F32, tag="dnrow")
                nc.vector.tensor_copy(out=dnrow, in_=acc[HD:HD + 1, :])
                dn = rcp.tile([128, 4], F32, tag="dn")
                nc.sync.dma_start(dn, dnrow)
                rc = rcp.tile([128, 4], F32, tag="rc")
                nc.vector.reciprocal(rc, dn)
                nc.sync.dma_start(_r(rpad[0:1, :]), _r(rc))
                bc = psb.tile([HD, 512], F32)
                nc.tensor.matmul(bc, _r(ones_sb), _r(rpad), start=True, stop=True)
                bcs = nrm.tile([HD, 512], F32, tag="bcs")
                nc.any.tensor_copy(out=bcs, in_=bc)
                onorm = nrm.tile([HD, 512], F32, tag="onorm")
                nc.vector.tensor_tensor(onorm, acc[0:HD, :], bcs, MULT)
                # stage into a2a input: shard j = token-slice owner core
                g0 = b * S + qt * 512
                j = g0 // TSLICE
                off = g0 % TSLICE
                nc.sync.dma_start(a2a_in.ap()[j * HD:(j + 1) * HD, off:off + 512], onorm)

        # ---- phase 3: all-to-all (head-major -> token-sliced channel-major) ----
        nc.gpsimd.collective_compute(
            "AllToAll", mybir.AluOpType.bypass,
            replica_groups=[list(range(NCORES))],
            ins=[a2a_in.ap().opt()],
            outs=[a2a_out.ap().opt()],
        )

        # ---- phase 4: output projection y.T = Wproj @ out_all.T + bias ----
        agc = const.tile([128, KC, TSLICE], F32)
        nc.sync.dma_start(_r(agc), _r(a2a_out.ap().rearrange("(kc p) t -> p kc t", p=128)))
        for ko in range(KC):
            y_sb = yp.tile([128, TSLICE], F32)
            for nt in range(2):
                py = psc.tile([128, 512], F32, tag="sc512")
                for kc in range(KC):
                    nc.tensor.matmul(
                        py, _r(wp_sb[:, kc, ko * 128:(ko + 1) * 128]),
                        _r(agc[:, kc, nt * 512:(nt + 1) * 512]),
                        start=(kc == 0), stop=(kc == KC - 1),
                    )
                nc.scalar.activation(
                    y_sb[:, nt * 512:(nt + 1) * 512], py, IDENT,
                    bias=bias_sb[:, ko:ko + 1], scale=1.0,
                )
            nc.sync.dma_start(outd.ap()[ko * 128:(ko + 1) * 128, :], y_sb)

    nc.compile()
    return nc


_NC_CACHE = None


def _get_nc():
    global _NC_CACHE
    if _NC_CACHE is None:
        _NC_CACHE = build_nc()
    return _NC_CACHE


def make_in_maps(x, cos, sin, Wqkv, Wproj, bproj):
    x = np.asarray(x, np.float32)
    cos = np.ascontiguousarray(np.asarray(cos, np.float32))
    sin = np.asarray(sin, np.float32)
    Wqkv = np.asarray(Wqkv, np.float32)
    Wproj = np.asarray(Wproj, np.float32)
    bproj = np.asarray(bproj, np.float32)

    xT = np.ascontiguousarray(x.reshape(T, C).T)           # [C, T]
    wprojT = np.ascontiguousarray(Wproj.T)                 # [C_in, C_out]
    s = sin.copy()
    s[:, 0::2] = -sin[:, 0::2]
    s = np.ascontiguousarray(s)
    bias2 = np.ascontiguousarray(bproj.reshape(KC, 128).T)  # [128, KC]

    in_maps = []
    for h in range(NCORES):
        wh = np.concatenate(
            [
                Wqkv[h * HD:(h + 1) * HD],                 # q rows
                Wqkv[C + h * HD:C + (h + 1) * HD],         # k rows
                Wqkv[2 * C + h * HD:2 * C + (h + 1) * HD], # v rows
            ],
            axis=0,
        )                                                  # [3*HD, C]
        wqkvT_h = np.ascontiguousarray(wh.T)               # [C, 3*HD]
        in_maps.append({
            "xT": xT,
            "wqkvT": wqkvT_h,
            "wprojT": wprojT,
            "cos": cos,
            "s": s,
            "bias": bias2,
        })
    return in_maps


def assemble_output(results):
    y = np.empty((T, C), np.float32)
    for h in range(NCORES):
        y[h * TSLICE:(h + 1) * TSLICE, :] = results[h]["out"].T
    return y.reshape(B, S, C)


def kernel(x, cos, sin, Wqkv, Wproj, bproj, _trace=False, **run_kwargs):
    nc = _get_nc()
    in_maps = make_in_maps(x, cos, sin, Wqkv, Wproj, bproj)
    res = run_bass_kernel_spmd(
        nc, in_maps, core_ids=list(range(NCORES)), trace=_trace, **run_kwargs
    )
    out = assemble_output(res.results)
    kernel.last_results = res
    return out


if __name__ == "__main__":
    nc = build_nc()
    print("built OK, instructions:", len(nc.inst_map))
